# revision 1
# baseline (speedup 1.0000x reference)
"""ARAP loss kernel for Trainium2 (8 NeuronCores, Bass/Tile).

Mathematical reformulation (exact):
  reference loss = sum_n sum_k w (d - R_n r)^2  with R_n from SVD of
  S_n = sum_k (w r)_k d_k^T, R = V U^T. Since tr(R S) = sum of singular
  values (nuclear norm),
      loss = E1 - 2 * sum_n nuc(S_n),   E1 = sum_{n,k} w (|d|^2 + |r|^2).

Structure exploited (verified at runtime against elem_idx):
  * The mesh is the deterministic 512x512 grid of reference.py: the
    gather V[elem_idx] is a fixed stencil.
  * Each face's 3 edges are replicated to its 3 vertices with identical
    weights => per-vertex element lists collapse 3x to face-major form:
    S_n = sum_{f ni n} M_f with one shared 3x3 M_f per face, and
    E1 = 3 * sum_f e_f.
  * Triangle closure (r2 = -(r0+r1), d2 = -(d0+d1)) collapses the three
    outer products per face to two:  M_f = g0 d0^T + g1 d1^T  with
    g0 = (w0+w2) r0 + w2 r1,  g1 = (w1+w2) r1 + w2 r0.

Device pipeline per core (64 vertex-row band, partition dim = grid col j):
  per j-tile (4 x 128 partitions):
    d vectors from shifted V reads (DVE) -> M_L, M_U outer products (DVE)
    -> 6-term S stencil as identity/shift matmuls accumulated in PSUM (PE)
    -> A = S^T S components (ACT squares + DVE) -> A_all buffer
    -> e_d weighted quadratic per cell (ACT + DVE) -> e_all buffer
  once: closed-form eigenvalues of A (trig method, ACT Sqrt/Sin/Arctan),
    nuc = sum sqrt(eig), fused per-partition reductions -> out [128, 2].
Host: loss = 3*(sum e_d + sum e_r) - 2*sum nuc   (e_r is static data only).
"""

import numpy as np

import concourse.bacc as bacc
import concourse.bass as bass
import concourse.mybir as mybir
import concourse.tile as tile
from concourse.bass_utils import run_bass_kernel_spmd

F32 = mybir.dt.float32
AF = mybir.ActivationFunctionType
OP = mybir.AluOpType

GRID = 512
CORES = 8
STAGE = 99  # debug: truncate device program


# ---------------------------------------------------------------------------
# host-side index structure (deterministic for the fixed grid)
# ---------------------------------------------------------------------------

def _grid_faces(n):
    idx = np.arange(n * n).reshape(n, n)
    v00 = idx[:-1, :-1].ravel(); v01 = idx[:-1, 1:].ravel()
    v10 = idx[1:, :-1].ravel(); v11 = idx[1:, 1:].ravel()
    F = np.concatenate(
        [np.stack([v00, v10, v11], 1), np.stack([v00, v11, v01], 1)], 0)
    return F


def _elem_maps(n):
    """(verts_s, pos, inv_order) of the reference element-list construction."""
    F = _grid_faces(n)
    verts = np.tile(F, (1, 3)).ravel()
    order = np.argsort(verts, kind='stable')
    verts_s = verts[order]
    counts = np.bincount(verts, minlength=n * n)
    starts = np.cumsum(counts) - counts
    pos = np.arange(verts.size) - np.repeat(starts, counts)
    inv = np.empty_like(order)
    inv[order] = np.arange(order.size)
    return F, verts_s, pos, inv


def _structure_ok(elem_idx, n):
    F, verts_s, pos, _ = _elem_maps(n)
    K = elem_idx.shape[1]
    es = np.repeat(F[:, [0, 1, 2]], 3, axis=1).ravel()
    et = np.repeat(F[:, [1, 2, 0]], 3, axis=1).ravel()
    rec = np.zeros((n * n, K, 2), dtype=elem_idx.dtype)
    order = np.argsort(np.tile(F, (1, 3)).ravel(), kind='stable')
    rec[verts_s, pos, 0] = es[order]
    rec[verts_s, pos, 1] = et[order]
    return np.array_equal(rec, np.asarray(elem_idx))


def _reference_fallback(V, elem_rest, elem_weights, elem_idx):
    """Exact numpy replica of the reference for unexpected inputs."""
    d = V[elem_idx[:, :, 1]] - V[elem_idx[:, :, 0]]
    w = elem_weights[:, :, None]
    S = np.einsum('nki,nkj->nij', elem_rest * w, d)
    U, _, Vt = np.linalg.svd(S)
    R = np.einsum('nji,nkj->nik', Vt, U)
    rest_rot = np.einsum('nij,nkj->nki', R, elem_rest)
    diff = d - rest_rot
    return np.asarray(np.sum(diff ** 2 * w), dtype=np.float32)


# ---------------------------------------------------------------------------
# host-side data prep
# ---------------------------------------------------------------------------

def _host_prep(V, elem_rest, elem_weights, grid=GRID, cores=CORES):
    n = grid
    ncell = n - 1
    rpc = n // cores          # vertex rows per core
    ci = rpc + 1              # cell rows per core incl. halo
    fhalf = ncell * ncell

    _, verts_s, pos, inv = _elem_maps(n)
    w9 = elem_weights[verts_s, pos][inv].reshape(-1, 9)
    r9 = elem_rest[verts_s, pos][inv].reshape(-1, 9, 3)
    wF = np.ascontiguousarray(w9[:, ::3])            # [Fn, 3]
    rF = np.ascontiguousarray(r9[:, ::3])            # [Fn, 3, 3]

    w0, w1, w2 = wF[:, 0], wF[:, 1], wF[:, 2]
    r0, r1, r2 = rF[:, 0], rF[:, 1], rF[:, 2]
    g0 = (w0 + w2)[:, None] * r0 + w2[:, None] * r1  # [Fn, 3]
    g1 = (w1 + w2)[:, None] * r1 + w2[:, None] * r0
    a = w0 + w2
    b = w1 + w2
    c2 = 2.0 * w2
    e_r_total = float(
        (w0.astype(np.float64) * (r0.astype(np.float64) ** 2).sum(1)
         + w1.astype(np.float64) * (r1.astype(np.float64) ** 2).sum(1)
         + w2.astype(np.float64) * (r2.astype(np.float64) ** 2).sum(1)).sum())

    def grd(x):  # [Fn/2, ...] lower/upper face grid [ncell, ncell, ...]
        return x.reshape(ncell, ncell, *x.shape[1:])

    # global per-cell feature grid: [cellrow + 1, jc, 18]
    q = np.zeros((n + 1, n, 18), np.float32)
    rows = slice(1, ncell + 1)
    cols = slice(0, ncell)
    q[rows, cols, 0:3] = grd(g0[:fhalf])
    q[rows, cols, 3:6] = grd(g1[:fhalf])
    q[rows, cols, 6:9] = grd(g0[fhalf:])
    q[rows, cols, 9:12] = grd(g1[fhalf:])
    q[rows, cols, 12] = grd(a[:fhalf])
    q[rows, cols, 13] = grd(b[:fhalf])
    q[rows, cols, 14] = grd(a[fhalf:])
    q[rows, cols, 15] = grd(b[fhalf:])
    q[rows, cols, 16] = grd(c2[:fhalf])
    q[rows, cols, 17] = grd(c2[fhalf:])

    vglob = np.zeros((n + 2, n, 3), np.float32)
    vglob[1:n + 1] = V.reshape(n, n, 3)

    g_maps = []
    v_maps = []
    for c in range(cores):
        gc = q[c * rpc: c * rpc + ci]                    # [ci, n, 18]
        g_maps.append(np.ascontiguousarray(gc.transpose(1, 2, 0)))  # [n,18,ci]
        vc = vglob[c * rpc: c * rpc + ci + 1]            # [ci+1, n, 3]
        vi = np.zeros((n + 1, 3, ci + 1), np.float32)
        vi[:n] = vc.transpose(1, 2, 0)                   # [n, 3, ci+1]
        v_maps.append(vi)

    return g_maps, v_maps, e_r_total


def _shift_mats(pj):
    m = np.zeros((pj, 3, pj), np.float32)
    m[:, 0, :] = np.eye(pj, dtype=np.float32)                # identity
    m[np.arange(pj - 1), 1, np.arange(1, pj)] = 1.0          # out[m]=rhs[m-1]
    m[pj - 1, 2, 0] = 1.0                                    # out[0]=rhs[pj-1]
    return m


# ---------------------------------------------------------------------------
# device program
# ---------------------------------------------------------------------------

def build_bass(grid=GRID, cores=CORES):
    n = grid
    rpc = n // cores
    ci = rpc + 1
    vi = rpc + 2
    pj = min(128, n)
    njt = n // pj
    ni = njt * rpc            # eig free size per comp

    nc = bacc.Bacc("TRN2", target_bir_lowering=False, debug=False,
                   enable_asserts=False)
    v_in = nc.dram_tensor("vtx", [n + 1, 3, vi], F32, kind="ExternalInput")
    g_in = nc.dram_tensor("gfc", [n, 18, ci], F32, kind="ExternalInput")
    m_in = nc.dram_tensor("mats", [pj, 3, pj], F32, kind="ExternalInput")
    out = nc.dram_tensor("out", [pj, 2], F32, kind="ExternalOutput")

    with tile.TileContext(nc) as tc:
        _emit(tc, v_in.ap(), g_in.ap(), m_in.ap(), out.ap(),
              n, rpc, ci, vi, pj, njt, ni)
    nc.compile()
    return nc


def _emit(tc, v_in, g_in, m_in, out, n, rpc, ci, vi, pj, njt, ni):
    from contextlib import ExitStack
    nc = tc.nc
    ctx = ExitStack()
    with ctx:
        singles = ctx.enter_context(tc.tile_pool(name="singles", bufs=1))
        stream = ctx.enter_context(tc.tile_pool(name="stream", bufs=2))
        work = ctx.enter_context(tc.tile_pool(name="work", bufs=2))
        mwork = ctx.enter_context(tc.tile_pool(name="mwork", bufs=2))
        psum = ctx.enter_context(tc.tile_pool(name="psum", bufs=2, space="PSUM"))
        ph2 = ctx.enter_context(tc.tile_pool(name="ph2", bufs=1))

        mats = singles.tile([pj, 3, pj], F32, name="mats", tag="mats")
        nc.sync.dma_start(out=mats, in_=m_in)
        ident = mats[:, 0, :]
        shm = mats[:, 1, :]
        e127 = mats[:, 2, :]

        # bias tiles for ACT ops (no ConstAPDatabase in this context)
        bias0 = singles.tile([pj, 1], F32, name="bias0", tag="bias0")
        nc.vector.memset(bias0, 0.0)
        sinb = singles.tile([pj, 3], F32, name="sinb", tag="sinb")
        for k, bv in enumerate((np.pi / 2, -np.pi / 6, -np.pi / 6)):
            nc.vector.memset(sinb[:, k:k + 1], float(bv))

        a_all = singles.tile([pj, 6, ni], F32, name="a_all", tag="a_all")
        e_all = singles.tile([pj, njt * rpc], F32, name="e_all", tag="e_all")
        outp = singles.tile([pj, 2], F32, name="outp", tag="outp")

        ml_prev = None
        mu_prev = None
        for t in range(njt):
            jlo = t * pj
            gt = stream.tile([pj, 18, ci], F32, name="gt", tag="gt")
            nc.sync.dma_start(out=gt, in_=g_in[jlo:jlo + pj])
            vt = stream.tile([pj, 3, vi], F32, name="vt", tag="vt")
            nc.sync.dma_start(out=vt, in_=v_in[jlo:jlo + pj])
            vs = stream.tile([pj, 3, vi], F32, name="vs", tag="vs")
            nc.sync.dma_start(out=vs, in_=v_in[jlo + 1:jlo + pj + 1])

            # --- d vectors [pj, 12, ci]: dL0, dL1, dU0, dU1 -------------
            if STAGE < 1:
                continue
            d = work.tile([pj, 12, ci], F32, name="d", tag="d")
            v0 = vt[:, :, 0:ci]        # V(i, j)
            v1 = vt[:, :, 1:ci + 1]    # V(i+1, j)
            s0 = vs[:, :, 0:ci]        # V(i, j+1)
            s1 = vs[:, :, 1:ci + 1]    # V(i+1, j+1)
            nc.vector.tensor_sub(d[:, 0:3, :], v1, v0)    # dL0
            nc.vector.tensor_sub(d[:, 3:6, :], s1, v1)    # dL1
            nc.vector.tensor_sub(d[:, 6:9, :], s1, v0)    # dU0
            nc.vector.tensor_sub(d[:, 9:12, :], s0, s1)   # dU1

            # --- M_L, M_U: g0 x d0 + g1 x d1  [pj, 9(a,b), ci] ----------
            def outer(gslice, dslice, dest, op):
                gin = gslice.rearrange('p a (b i) -> p a b i', b=1)\
                            .broadcast_to([pj, 3, 3, ci])
                din = dslice.rearrange('p (a b) i -> p a b i', a=1)\
                            .broadcast_to([pj, 3, 3, ci])
                do = dest.rearrange('p (a b) i -> p a b i', a=3)
                nc.vector.tensor_tensor(do, gin, din, op)

            if STAGE < 2:
                continue
            ml = mwork.tile([pj, 9, ci], F32, name="ml", tag="ml")
            mu = mwork.tile([pj, 9, ci], F32, name="mu", tag="mu")
            tmp = work.tile([pj, 9, ci], F32, name="tmp", tag="tmp")
            outer(gt[:, 0:3, :], d[:, 0:3, :], tmp, OP.mult)
            outer(gt[:, 3:6, :], d[:, 3:6, :], ml, OP.mult)
            nc.vector.tensor_add(ml, ml, tmp)
            outer(gt[:, 6:9, :], d[:, 6:9, :], tmp, OP.mult)
            outer(gt[:, 9:12, :], d[:, 9:12, :], mu, OP.mult)
            nc.vector.tensor_add(mu, mu, tmp)

            # --- S stencil on PE: S(iv,j) = ML(i)+MU(i)+ML(i-1)
            #                               +[MU(i)+ML(i-1)+MU(i-1)](j-1) ---
            if STAGE < 3:
                ml_prev, mu_prev = ml, mu
                continue
            s_ps = psum.tile([pj, 9 * rpc], F32, name="s_ps", tag="s_ps")
            bank = 512  # fp32 slots per PSUM bank
            terms = [(ident, ml, 1), (ident, mu, 1), (ident, ml, 0),
                     (shm, mu, 1), (shm, ml, 0), (shm, mu, 0)]
            if t > 0:
                terms += [(e127, mu_prev, 1), (e127, ml_prev, 0),
                          (e127, mu_prev, 0)]
            nq0 = min(9, bank // rpc)      # comps in bank0
            for lo, cnt in ((0, nq0), (nq0, 9 - nq0)):
                if cnt == 0:
                    continue
                for k, (lhs, m_t, off) in enumerate(terms):
                    rhs = m_t[:, lo:lo + cnt, off:off + rpc]
                    o = s_ps[:, lo * rpc:(lo + cnt) * rpc]\
                        .rearrange('p (q i) -> p q i', q=cnt)
                    nc.tensor.matmul(o, lhs, rhs,
                                     start=(k == 0), stop=(k == len(terms) - 1))

            if STAGE < 4:
                ml_prev, mu_prev = ml, mu
                continue
            ssb = work.tile([pj, 9, rpc], F32, name="ssb", tag="ssb")
            nc.vector.tensor_copy(ssb.rearrange('p q i -> p (q i)'), s_ps)

            # --- A = S^T S  (6 comps) into a_all[:, :, t*rpc:...] --------
            if STAGE < 5:
                ml_prev, mu_prev = ml, mu
                continue
            sq = work.tile([pj, 9, rpc], F32, name="sq", tag="sq")
            nc.scalar.activation(sq, ssb, AF.Square, bias=bias0)
            isl = slice(t * rpc, (t + 1) * rpc)
            # diag: A[c,c] = sum_r sq[3r+c]
            dtmp = work.tile([pj, 3, rpc], F32, name="dtmp", tag="dtmp")
            nc.vector.tensor_add(dtmp, sq[:, 0:3, :], sq[:, 3:6, :])
            nc.vector.tensor_add(a_all[:, 0:3, isl], dtmp, sq[:, 6:9, :])
            # off-diag products
            pra = work.tile([pj, 3, 2, rpc], F32, name="pra", tag="pra")
            s3 = ssb.rearrange('p (r c) i -> p r c i', c=3)
            nc.vector.tensor_tensor(
                pra, s3[:, :, 0:1, :].broadcast_to([pj, 3, 2, rpc]),
                s3[:, :, 1:3, :], OP.mult)
            prb = work.tile([pj, 3, rpc], F32, name="prb", tag="prb")
            nc.vector.tensor_tensor(prb, s3[:, :, 1, :], s3[:, :, 2, :], OP.mult)
            otmp = work.tile([pj, 2, rpc], F32, name="otmp", tag="otmp")
            nc.vector.tensor_add(otmp, pra[:, 0], pra[:, 1])
            nc.vector.tensor_add(a_all[:, 3:5, isl], otmp, pra[:, 2])
            otmp2 = work.tile([pj, 1, rpc], F32, name="otmp2", tag="otmp2")
            nc.vector.tensor_add(otmp2[:, 0], prb[:, 0], prb[:, 1])
            nc.vector.tensor_add(a_all[:, 5, isl], otmp2[:, 0], prb[:, 2])

            # --- e_d per cell ------------------------------------------
            if STAGE < 6:
                ml_prev, mu_prev = ml, mu
                continue
            sqd = work.tile([pj, 12, ci], F32, name="sqd", tag="sqd")
            nc.scalar.activation(sqd, d, AF.Square, bias=bias0)
            cr = work.tile([pj, 2, 3, ci], F32, name="cr", tag="cr")
            d4 = d.rearrange('p (pr e c) i -> p pr e c i', pr=2, e=2)
            nc.vector.tensor_tensor(cr, d4[:, :, 0], d4[:, :, 1], OP.mult)
            tb = work.tile([pj, ci, 18], F32, name="tb", tag="tb")
            w1o = tb[:, :, 0:12].rearrange('p i (e c) -> p e c i', c=3)
            nc.vector.tensor_tensor(
                w1o, sqd.rearrange('p (e c) i -> p e c i', c=3),
                gt[:, 12:16, :].rearrange('p e (b i) -> p e b i', b=1)
                  .broadcast_to([pj, 4, 3, ci]), OP.mult)
            w2o = tb[:, :, 12:18].rearrange('p i (e c) -> p e c i', c=3)
            nc.vector.tensor_tensor(
                w2o, cr,
                gt[:, 16:18, :].rearrange('p e (b i) -> p e b i', b=1)
                  .broadcast_to([pj, 2, 3, ci]), OP.mult)
            # skip ci=0 (the halo row; counted by the neighboring core)
            nc.vector.tensor_reduce(
                e_all[:, t * rpc:(t + 1) * rpc], tb[:, 1:ci, :],
                mybir.AxisListType.X, OP.add)

            ml_prev, mu_prev = ml, mu

        # ---- phase 2: eigenvalues + nuclear norm on a_all --------------
        def t1(tag):
            return ph2.tile([pj, ni], F32, name=tag, tag=tag)

        if STAGE < 7:
            nc.vector.memset(outp, 0.0)
            nc.sync.dma_start(out=out, in_=outp)
            return

        a0, a1, a2 = a_all[:, 0, :], a_all[:, 1, :], a_all[:, 2, :]
        o01, o02, o12 = a_all[:, 3, :], a_all[:, 4, :], a_all[:, 5, :]

        q3 = t1("q3")
        nc.vector.tensor_add(q3, a0, a1)
        nc.vector.tensor_add(q3, q3, a2)
        qv = t1("qv")
        nc.vector.tensor_scalar_mul(qv, q3, 1.0 / 3.0)
        bd = ph2.tile([pj, 3, ni], F32, name="bd", tag="bd")
        nc.vector.tensor_tensor(
            bd, a_all[:, 0:3, :],
            qv.rearrange('p (c i) -> p c i', c=1).broadcast_to([pj, 3, ni]),
            OP.subtract)
        sq6 = ph2.tile([pj, 6, ni], F32, name="sq6", tag="sq6")
        nc.scalar.activation(sq6[:, 0:3, :], bd, AF.Square, bias=bias0)
        nc.scalar.activation(sq6[:, 3:6, :], a_all[:, 3:6, :], AF.Square, bias=bias0)
        sd = t1("sd")
        nc.vector.tensor_add(sd, sq6[:, 0, :], sq6[:, 1, :])
        nc.vector.tensor_add(sd, sd, sq6[:, 2, :])
        so = t1("so")
        nc.vector.tensor_add(so, sq6[:, 3, :], sq6[:, 4, :])
        nc.vector.tensor_add(so, so, sq6[:, 5, :])
        p2 = t1("p2")
        nc.vector.scalar_tensor_tensor(p2, so, 2.0, sd, OP.mult, OP.add)

        b0, b1, b2 = bd[:, 0, :], bd[:, 1, :], bd[:, 2, :]
        x1 = t1("x1"); x2 = t1("x2"); det = t1("det")
        nc.vector.tensor_mul(x1, b1, b2)
        nc.vector.tensor_sub(x1, x1, sq6[:, 5, :])
        nc.vector.tensor_mul(det, b0, x1)                 # T1
        nc.vector.tensor_mul(x1, o01, b2)
        nc.vector.tensor_mul(x2, o12, o02)
        nc.vector.tensor_sub(x1, x1, x2)
        nc.vector.tensor_mul(x1, o01, x1)                 # T2
        nc.vector.tensor_sub(det, det, x1)
        nc.vector.tensor_mul(x1, o01, o12)
        nc.vector.tensor_mul(x2, b1, o02)
        nc.vector.tensor_sub(x1, x1, x2)
        nc.vector.tensor_mul(x1, o02, x1)                 # T3
        nc.vector.tensor_add(det, det, x1)

        if STAGE < 8:
            nc.vector.memset(outp, 0.0)
            nc.vector.tensor_reduce(outp[:, 0:1], det, mybir.AxisListType.X, OP.add)
            nc.sync.dma_start(out=out, in_=outp)
            return
        sv = t1("sv")
        nc.vector.tensor_scalar_mul(sv, p2, 1.0 / 6.0)    # s = p2/6
        s2 = t1("s2")
        nc.vector.tensor_mul(s2, sv, sv)
        s4 = t1("s4")
        nc.vector.tensor_scalar_mul(s4, sv, 4.0)
        u = t1("u")
        nc.vector.tensor_mul(u, s2, s4)                   # 4 p^6
        det2 = t1("det2")
        nc.scalar.activation(det2, det, AF.Square, bias=bias0)
        nc.vector.tensor_sub(u, u, det2)                  # 4p^6 - det^2
        nc.vector.tensor_scalar_max(u, u, 1e-30)
        ru = t1("ru")
        nc.vector.reciprocal(ru, u)
        rs = t1("rs")
        nc.scalar.activation(rs, ru, AF.Sqrt, bias=bias0)             # 1/sqrt(4p^6-det^2)
        pv = t1("pv")
        nc.scalar.activation(pv, sv, AF.Sqrt, bias=bias0)             # p = sqrt(p2/6)
        if STAGE < 9:
            nc.vector.memset(outp, 0.0)
            nc.vector.tensor_reduce(outp[:, 0:1], rs, mybir.AxisListType.X, OP.add)
            nc.vector.tensor_reduce(outp[:, 1:2], pv, mybir.AxisListType.X, OP.add)
            nc.sync.dma_start(out=out, in_=outp)
            return
        arg = t1("arg")
        nc.vector.tensor_mul(arg, det, rs)       # rc/sqrt(1-rc^2), unbounded
        # range-reduced arctan (ACT Arctan domain is [-pi/2, pi/2]):
        # atan(|a|) = atan(min(|a|, 1/|a|)), flipped across pi/4 for |a|>1.
        sgn = t1("sgn")
        nc.vector.tensor_scalar(sgn, arg, 0.0, 2.0, OP.is_ge, OP.mult)
        nc.vector.tensor_scalar_add(sgn, sgn, -1.0)          # sign(arg)
        aa = t1("aa")
        nc.vector.tensor_mul(aa, arg, sgn)                   # |arg|
        nc.vector.tensor_scalar_max(aa, aa, 1e-20)           # keep 1/aa finite
        iaa = t1("iaa")
        nc.vector.reciprocal(iaa, aa)
        mm = t1("mm")
        nc.vector.tensor_tensor(mm, aa, iaa, OP.min)
        if STAGE < 9.2:
            nc.vector.memset(outp, 0.0)
            nc.vector.tensor_reduce(outp[:, 0:1], mm, mybir.AxisListType.X, OP.add)
            nc.sync.dma_start(out=out, in_=outp)
            return
        at = t1("at")
        nc.scalar.activation(at, mm, AF.Arctan, bias=bias0)  # in [0, pi/4]
        if STAGE < 9.4:
            nc.vector.memset(outp, 0.0)
            nc.vector.tensor_reduce(outp[:, 0:1], at, mybir.AxisListType.X, OP.add)
            nc.sync.dma_start(out=out, in_=outp)
            return
        flip = t1("flip")
        nc.vector.tensor_scalar(flip, aa, 1.0, 1.0, OP.is_gt, OP.mult)
        f2 = t1("f2")
        nc.vector.tensor_scalar(f2, at, -2.0, float(np.pi / 2), OP.mult, OP.add)
        nc.vector.tensor_mul(flip, flip, f2)
        nc.vector.tensor_add(at, at, flip)                   # atan(|arg|)
        nc.vector.tensor_mul(at, at, sgn)                    # atan(arg)
        if STAGE < 9.6:
            nc.vector.memset(outp, 0.0)
            nc.vector.tensor_reduce(outp[:, 0:1], at, mybir.AxisListType.X, OP.add)
            nc.sync.dma_start(out=out, in_=outp)
            return
        th = t1("th")
        nc.scalar.activation(th, at, AF.Copy,
                             bias=float(np.pi / 6), scale=-1.0 / 3.0)
        if STAGE < 10:
            nc.vector.memset(outp, 0.0)
            nc.vector.tensor_reduce(outp[:, 0:1], th, mybir.AxisListType.X, OP.add)
            nc.sync.dma_start(out=out, in_=outp)
            return
        tp = t1("tp")
        nc.vector.tensor_scalar_mul(tp, pv, 2.0)
        lam = ph2.tile([pj, 3, ni], F32, name="lam", tag="lam")
        # cos(th) = sin(th + pi/2); cos(th - 2pi/3) = sin(th - pi/6);
        # cos(th + 2pi/3) = sin(-th - pi/6)  (keeps Sin args in [-pi, pi])
        sin_forms = ((1.0, np.pi / 2), (1.0, -np.pi / 6), (-1.0, -np.pi / 6))
        cs = t1("cs")
        for k, (sc, sh) in enumerate(sin_forms):
            nc.scalar.activation(cs, th, AF.Sin,
                                 bias=sinb[:, k:k + 1], scale=sc)
            nc.vector.tensor_mul(lam[:, k, :], tp, cs)
        nc.vector.tensor_tensor(
            lam, lam,
            qv.rearrange('p (c i) -> p c i', c=1).broadcast_to([pj, 3, ni]),
            OP.add)
        nc.vector.tensor_scalar_max(lam, lam, 0.0)
        sg = ph2.tile([pj, 3, ni], F32, name="sg", tag="sg")
        nc.scalar.activation(sg, lam, AF.Sqrt, bias=bias0)
        n1 = t1("n1")
        nc.vector.tensor_add(n1, sg[:, 0, :], sg[:, 1, :])
        nc.vector.tensor_add(n1, n1, sg[:, 2, :])
        nc.vector.tensor_reduce(outp[:, 0:1], n1, mybir.AxisListType.X, OP.add)
        nc.vector.tensor_reduce(
            outp[:, 1:2], e_all, mybir.AxisListType.X, OP.add)

        nc.sync.dma_start(out=out, in_=outp)


# ---------------------------------------------------------------------------
# entry point
# ---------------------------------------------------------------------------

_NC_CACHE = {}


def _get_nc(grid=GRID, cores=CORES):
    key = (grid, cores)
    if key not in _NC_CACHE:
        _NC_CACHE[key] = build_bass(grid, cores)
    return _NC_CACHE[key]


def run_device(g_maps, v_maps, grid=GRID, cores=CORES, trace=False):
    nc = _get_nc(grid, cores)
    pj = min(128, grid)
    mats = _shift_mats(pj)
    in_maps = [{"vtx": v_maps[c], "gfc": g_maps[c], "mats": mats}
               for c in range(cores)]
    res = run_bass_kernel_spmd(nc, in_maps, core_ids=list(range(cores)),
                               trace=trace)
    return res


def kernel(V_deformed, elem_rest, elem_weights, elem_idx):
    V = np.asarray(V_deformed, np.float32)
    er = np.asarray(elem_rest, np.float32)
    ew = np.asarray(elem_weights, np.float32)
    ei = np.asarray(elem_idx)
    n = GRID
    assert V.shape == (n * n, 3)

    if not _structure_ok(ei, n):
        return _reference_fallback(V, er, ew, ei)

    g_maps, v_maps, e_r_total = _host_prep(V, er, ew, n, CORES)
    res = run_device(g_maps, v_maps, n, CORES)
    nuc_sum = 0.0
    e_sum = 0.0
    for r in res.results:
        o = r["out"].astype(np.float64)
        nuc_sum += o[:, 0].sum()
        e_sum += o[:, 1].sum()
    loss = 3.0 * (e_sum + e_r_total) - 2.0 * nuc_sum
    return np.asarray(loss, dtype=np.float32)



# revision 13
# speedup vs baseline: 1.2151x; 1.2151x over previous
"""ARAP loss kernel for Trainium2 (8 NeuronCores, Bass/Tile).

Mathematical reformulation (exact):
  reference loss = sum_n sum_k w (d - R_n r)^2  with R_n from SVD of
  S_n = sum_k (w r)_k d_k^T, R = V U^T. Since tr(R S) = sum of singular
  values (nuclear norm),
      loss = E1 - 2 * sum_n nuc(S_n),   E1 = sum_{n,k} w (|d|^2 + |r|^2).

Structure exploited (verified at runtime against elem_idx):
  * The mesh is the deterministic 512x512 grid of reference.py: the
    gather V[elem_idx] is a fixed stencil.
  * Each face's 3 edges are replicated to its 3 vertices with identical
    weights => per-vertex element lists collapse 3x to face-major form:
    S_n = sum_{f ni n} M_f with one shared 3x3 M_f per face, and
    E1 = 3 * sum_f e_f.
  * Triangle closure (r2 = -(r0+r1), d2 = -(d0+d1)) collapses the three
    outer products per face to two:  M_f = g0 d0^T + g1 d1^T  with
    g0 = (w0+w2) r0 + w2 r1,  g1 = (w1+w2) r1 + w2 r0.

Device layout (v2): partition p = column j div 4, jsub s = j mod 4, so
both stencil shifts are free-dim ops except the s=0 column, which takes
one small PE shift-matmul. Face-matrix middle layer runs in bf16
(validated: final rel err ~2.7e-3 vs 2e-2 budget); eigen chain in fp32.
Per-core work: outer products (DVE bf16) -> X/T/W stencil adds -> S ->
A = S^T S (bf16) -> closed-form eigenvalues (trig, fp32) -> nuc; e_d
via one fused tensor_tensor_reduce. Host: loss = 3*(e_d + e_r) - 2*nuc.
"""

import numpy as np
import ml_dtypes

import concourse.bacc as bacc
import concourse.bass as bass
import concourse.mybir as mybir
import concourse.tile as tile
from concourse.bass_utils import run_bass_kernel_spmd

F32 = mybir.dt.float32
BF16 = mybir.dt.bfloat16
AF = mybir.ActivationFunctionType
OP = mybir.AluOpType
NPBF = ml_dtypes.bfloat16

GRID = 512
CORES = 8


# ---------------------------------------------------------------------------
# host-side index structure (deterministic for the fixed grid)
# ---------------------------------------------------------------------------

def _grid_faces(n):
    idx = np.arange(n * n).reshape(n, n)
    v00 = idx[:-1, :-1].ravel(); v01 = idx[:-1, 1:].ravel()
    v10 = idx[1:, :-1].ravel(); v11 = idx[1:, 1:].ravel()
    F = np.concatenate(
        [np.stack([v00, v10, v11], 1), np.stack([v00, v11, v01], 1)], 0)
    return F


def _elem_maps(n):
    """(verts_s, pos, inv_order) of the reference element-list construction."""
    F = _grid_faces(n)
    verts = np.tile(F, (1, 3)).ravel()
    order = np.argsort(verts, kind='stable')
    verts_s = verts[order]
    counts = np.bincount(verts, minlength=n * n)
    starts = np.cumsum(counts) - counts
    pos = np.arange(verts.size) - np.repeat(starts, counts)
    inv = np.empty_like(order)
    inv[order] = np.arange(order.size)
    return F, verts_s, pos, inv


def _structure_ok(elem_idx, n):
    F, verts_s, pos, _ = _elem_maps(n)
    K = elem_idx.shape[1]
    es = np.repeat(F[:, [0, 1, 2]], 3, axis=1).ravel()
    et = np.repeat(F[:, [1, 2, 0]], 3, axis=1).ravel()
    rec = np.zeros((n * n, K, 2), dtype=elem_idx.dtype)
    order = np.argsort(np.tile(F, (1, 3)).ravel(), kind='stable')
    rec[verts_s, pos, 0] = es[order]
    rec[verts_s, pos, 1] = et[order]
    return np.array_equal(rec, np.asarray(elem_idx))


def _reference_fallback(V, elem_rest, elem_weights, elem_idx):
    """Exact numpy replica of the reference for unexpected inputs."""
    d = V[elem_idx[:, :, 1]] - V[elem_idx[:, :, 0]]
    w = elem_weights[:, :, None]
    S = np.einsum('nki,nkj->nij', elem_rest * w, d)
    U, _, Vt = np.linalg.svd(S)
    R = np.einsum('nji,nkj->nik', Vt, U)
    rest_rot = np.einsum('nij,nkj->nki', R, elem_rest)
    diff = d - rest_rot
    return np.asarray(np.sum(diff ** 2 * w), dtype=np.float32)


# ---------------------------------------------------------------------------
# host-side data prep
# ---------------------------------------------------------------------------

def _host_prep(V, elem_rest, elem_weights, grid=GRID, cores=CORES):
    n = grid
    ncell = n - 1
    rpc = n // cores          # vertex rows per core
    ci = rpc + 1              # cell rows per core incl. halo
    fhalf = ncell * ncell

    _, verts_s, pos, inv = _elem_maps(n)
    w9 = elem_weights[verts_s, pos][inv].reshape(-1, 9)
    r9 = elem_rest[verts_s, pos][inv].reshape(-1, 9, 3)
    wF = np.ascontiguousarray(w9[:, ::3])            # [Fn, 3]
    rF = np.ascontiguousarray(r9[:, ::3])            # [Fn, 3, 3]

    w0, w1, w2 = wF[:, 0], wF[:, 1], wF[:, 2]
    r0, r1, r2 = rF[:, 0], rF[:, 1], rF[:, 2]
    g0 = (w0 + w2)[:, None] * r0 + w2[:, None] * r1  # [Fn, 3]
    g1 = (w1 + w2)[:, None] * r1 + w2[:, None] * r0
    a = w0 + w2
    b = w1 + w2
    c2 = 2.0 * w2
    e_r_total = float(
        (w0.astype(np.float64) * (r0.astype(np.float64) ** 2).sum(1)
         + w1.astype(np.float64) * (r1.astype(np.float64) ** 2).sum(1)
         + w2.astype(np.float64) * (r2.astype(np.float64) ** 2).sum(1)).sum())

    def grd(x):  # [Fn/2, ...] lower/upper face grid [ncell, ncell, ...]
        return x.reshape(ncell, ncell, *x.shape[1:])

    # global per-cell feature grid: [cellrow + 1, jc, 18]
    q = np.zeros((n + 1, n, 18), np.float32)
    rows = slice(1, ncell + 1)
    cols = slice(0, ncell)
    q[rows, cols, 0:3] = grd(g0[:fhalf])
    q[rows, cols, 3:6] = grd(g1[:fhalf])
    q[rows, cols, 6:9] = grd(g0[fhalf:])
    q[rows, cols, 9:12] = grd(g1[fhalf:])
    q[rows, cols, 12] = grd(a[:fhalf])
    q[rows, cols, 13] = grd(b[:fhalf])
    q[rows, cols, 14] = grd(a[fhalf:])
    q[rows, cols, 15] = grd(b[fhalf:])
    q[rows, cols, 16] = grd(c2[:fhalf])
    q[rows, cols, 17] = grd(c2[fhalf:])

    vglob = np.zeros((n + 2, n, 3), np.float32)
    vglob[1:n + 1] = V.reshape(n, n, 3)

    g_maps = []
    v_maps = []
    for c in range(cores):
        gc = q[c * rpc: c * rpc + ci]                    # [ci, n, 18]
        g_maps.append(np.ascontiguousarray(
            gc.transpose(1, 2, 0)).astype(NPBF))         # [n,18,ci] bf16
        vc = vglob[c * rpc: c * rpc + ci + 1]            # [ci+1, n, 3]
        vi = np.zeros((n + 1, 3, ci + 1), np.float32)
        vi[:n] = vc.transpose(1, 2, 0)                   # [n, 3, ci+1]
        v_maps.append(vi)

    return g_maps, v_maps, e_r_total


def _shift_mat(pj):
    # out[m] = rhs[m-1] (out[0] = 0): lhs[p, p+1] = 1
    m = np.zeros((pj, 1, pj), np.float32)
    m[np.arange(pj - 1), 0, np.arange(1, pj)] = 1.0
    return m.astype(NPBF)


# ---------------------------------------------------------------------------
# device program
# ---------------------------------------------------------------------------

def build_bass(grid=GRID, cores=CORES):
    n = grid
    rpc = n // cores
    ci = rpc + 1              # 65
    vi = rpc + 2              # 66
    pj = 128
    sj = n // pj              # 4 jsub columns per partition

    nc = bacc.Bacc("TRN2", target_bir_lowering=False, debug=False,
                   enable_asserts=False)
    v_in = nc.dram_tensor("vtx", [n + 1, 3, vi], F32, kind="ExternalInput")
    g_in = nc.dram_tensor("gfc", [n, 18, ci], BF16, kind="ExternalInput")
    m_in = nc.dram_tensor("mats", [pj, 1, pj], BF16, kind="ExternalInput")
    out = nc.dram_tensor("out", [pj, 8], F32, kind="ExternalOutput")

    with tile.TileContext(nc) as tc:
        _emit(tc, v_in.ap(), g_in.ap(), m_in.ap(), out.ap(),
              n, rpc, ci, vi, pj, sj)
    nc.compile()
    return nc


def _emit(tc, v_in, g_in, m_in, out, n, rpc, ci, vi, pj, sj):
    from contextlib import ExitStack
    nc = tc.nc
    ni = sj * rpc             # 256 vertices per partition
    ctx = ExitStack()
    with ctx:
        sg = ctx.enter_context(tc.tile_pool(name="sg", bufs=1))
        psum = ctx.enter_context(tc.tile_pool(name="psum", bufs=1, space="PSUM"))

        def st(shape, dtype, tag):
            return sg.tile([pj] + shape, dtype, name=tag, tag=tag)

        # ---- inputs -------------------------------------------------
        mats = st([1, pj], BF16, "mats")
        nc.sync.dma_start(out=mats, in_=m_in)
        vt = st([sj, 3, vi], F32, "vt")
        nc.sync.dma_start(
            out=vt, in_=v_in[0:n].rearrange('(p s) c i -> p s c i', s=sj))
        vs = st([sj, 3, vi], F32, "vs")
        nc.sync.dma_start(
            out=vs, in_=v_in[1:n + 1].rearrange('(p s) c i -> p s c i', s=sj))
        gt = st([sj, 18, ci], BF16, "gt")
        nc.sync.dma_start(
            out=gt, in_=g_in.rearrange('(p s) c i -> p s c i', s=sj))

        # constant bias tiles for ACT ops
        bias0 = st([1], F32, "bias0")
        nc.vector.memset(bias0, 0.0)
        sinb = st([3], F32, "sinb")
        for k, bv in enumerate((2 * np.pi / 3, 0.0, -np.pi / 3)):
            nc.gpsimd.memset(sinb[:, k:k + 1], float(bv))

        # ---- d vectors [pj, sj, 12, ci] bf16 ------------------------
        d = st([sj, 12, ci], BF16, "d")
        v0 = vt[:, :, :, 0:ci]
        v1 = vt[:, :, :, 1:ci + 1]
        s0 = vs[:, :, :, 0:ci]
        s1 = vs[:, :, :, 1:ci + 1]
        nc.gpsimd.tensor_sub(d[:, :, 0:3, :], v1, v0)    # dL0
        nc.gpsimd.tensor_sub(d[:, :, 3:6, :], s1, v1)    # dL1
        nc.gpsimd.tensor_sub(d[:, :, 6:9, :], s1, v0)    # dU0
        nc.gpsimd.tensor_sub(d[:, :, 9:12, :], s0, s1)   # dU1

        # ---- face matrices ML, MU [pj, 9, sj, ci] bf16 (comp-major) -
        ml = st([9, sj, ci], BF16, "ml")
        mu = st([9, sj, ci], BF16, "mu")
        tmpo = st([9, sj, ci], BF16, "tmpo")

        def outer(dst, gc0, dc0):
            # dst[3a+b, s, i] = g[a, s, i] * d[b, s, i]  (TT is limited to
            # 3 free dims, so one instruction per a-component)
            din = d[:, :, dc0:dc0 + 3, :].rearrange('p s b i -> p b s i')
            for a in range(3):
                gin = gt[:, :, gc0 + a, :]\
                    .rearrange('p s i -> p () s i')\
                    .broadcast_to([pj, 3, sj, ci])
                nc.vector.tensor_tensor(
                    dst[:, 3 * a:3 * a + 3, :, :], gin, din, OP.mult)

        outer(tmpo, 0, 0)       # g0L x dL0
        outer(ml, 3, 3)         # g1L x dL1
        nc.vector.tensor_add(ml, ml, tmpo)
        outer(tmpo, 6, 6)       # g0U x dU0
        outer(mu, 9, 9)         # g1U x dU1
        nc.vector.tensor_add(mu, mu, tmpo)

        # ---- stencil: X = MU(i)+ML(i-1); T = ML(i)+X; W = X+MU(i-1) -
        xb = st([9, sj, rpc], BF16, "xb")
        tb = st([9, sj, rpc], BF16, "tb")
        wb = st([9, sj, rpc], BF16, "wb")
        nc.vector.tensor_add(xb, mu[:, :, :, 1:ci], ml[:, :, :, 0:rpc])
        nc.vector.tensor_add(tb, ml[:, :, :, 1:ci], xb)
        nc.vector.tensor_add(wb, xb, mu[:, :, :, 0:rpc])

        # ---- column shift: s0psum[q*rpc+i] = W[p-1, q, 3, i] on PE --
        s0ps = psum.tile([pj, 9 * rpc], F32, name="s0ps", tag="s0ps")
        bank = 512
        nq0 = bank // rpc       # 8 comps in bank 0
        for lo, cnt in ((0, nq0), (nq0, 9 - nq0)):
            o = s0ps[:, lo * rpc:(lo + cnt) * rpc]\
                .rearrange('p (q i) -> p q i', q=cnt)
            nc.tensor.matmul(o, mats[:, 0, :], wb[:, lo:lo + cnt, sj - 1, :],
                             start=True, stop=True)

        # ---- S [pj, 9, sj, rpc] bf16 --------------------------------
        sS = st([9, sj, rpc], BF16, "sS")
        nc.vector.tensor_add(sS[:, :, 1:sj, :], tb[:, :, 1:sj, :],
                             wb[:, :, 0:sj - 1, :])
        nc.vector.tensor_add(
            sS[:, :, 0, :], tb[:, :, 0, :],
            s0ps.rearrange('p (q i) -> p q i', q=9))

        # ---- e_d: squares + cross terms + fused weighted reduce -----
        edt = st([sj, 18, ci], BF16, "edt")
        # sqd: d^2 -> slots 0:12
        nc.vector.tensor_mul(edt[:, :, 0:12, :], d, d)
        # cross: dL0*dL1 -> 12:15, dU0*dU1 -> 15:18
        nc.vector.tensor_tensor(edt[:, :, 12:15, :], d[:, :, 0:3, :],
                                d[:, :, 3:6, :], OP.mult)
        nc.vector.tensor_tensor(edt[:, :, 15:18, :], d[:, :, 6:9, :],
                                d[:, :, 9:12, :], OP.mult)
        eds = st([sj, 18, rpc], BF16, "eds")
        outp = st([8], F32, "outp")
        nc.vector.memset(outp, 0.0)
        for s in range(sj):
            win = gt[:, s, 12:18, 1:ci]\
                .rearrange('p w i -> p w () i')\
                .broadcast_to([pj, 6, 3, rpc])
            nc.vector.tensor_tensor(
                eds[:, s].rearrange('p (w c) i -> p w c i', c=3),
                edt[:, s, :, 1:ci].rearrange('p (w c) i -> p w c i', c=3),
                win, OP.mult)
        nc.vector.tensor_reduce(outp[:, 2:3], eds,
                                mybir.AxisListType.XYZ, OP.add)

        # ---- A = S^T S (6 comps) bf16 -------------------------------
        sq = st([9, ni], BF16, "sq")
        sf = sS.rearrange('p q s i -> p q (s i)')
        nc.vector.tensor_mul(sq, sf, sf)
        a_all = st([6, ni], BF16, "a_all")
        nc.vector.tensor_add(a_all[:, 0:3, :], sq[:, 0:3, :], sq[:, 3:6, :])
        nc.vector.tensor_add(a_all[:, 0:3, :], a_all[:, 0:3, :], sq[:, 6:9, :])
        paw = st([9, ni], BF16, "paw")
        s3 = sf.rearrange('p (a c) x -> p a c x', a=3)
        # pra: S[a,0]*S[a,1], S[a,0]*S[a,2] -> paw[2a], paw[2a+1]
        nc.vector.tensor_tensor(
            paw[:, 0:6, :].rearrange('p (a k) x -> p a k x', a=3),
            s3[:, :, 0, :].rearrange('p a x -> p a () x')
              .broadcast_to([pj, 3, 2, ni]),
            s3[:, :, 1:3, :], OP.mult)
        # prb: S[a,1]*S[a,2] -> paw[6+a]
        nc.vector.tensor_tensor(
            paw[:, 6:9, :], s3[:, :, 1, :], s3[:, :, 2, :], OP.mult)
        nc.vector.tensor_add(
            a_all[:, 3:5, :],
            paw[:, 0:2, :], paw[:, 2:4, :])
        nc.vector.tensor_add(a_all[:, 3:5, :], a_all[:, 3:5, :],
                             paw[:, 4:6, :])
        nc.vector.tensor_add(a_all[:, 5, :], paw[:, 6, :], paw[:, 7, :])
        nc.vector.tensor_add(a_all[:, 5, :], a_all[:, 5, :], paw[:, 8, :])

        # ---- phase 2: eigenvalues + nuclear norm, 2 chunks ----------
        nch = 2
        fch = ni // nch

        def t2(tag, c, comps=None):
            shape = [fch] if comps is None else [comps, fch]
            return sg.tile([pj] + shape, F32, name=f"{tag}{c}", tag=f"{tag}{c}")

        chunks = []
        for c in range(nch):
            isl = slice(c * fch, (c + 1) * fch)
            A = a_all[:, :, isl]
            q3 = t2("q3", c)
            nc.gpsimd.tensor_add(q3, A[:, 0, :], A[:, 1, :])
            nc.gpsimd.tensor_add(q3, q3, A[:, 2, :])
            qv = t2("qv", c)
            nc.gpsimd.tensor_scalar_mul(qv, q3, 1.0 / 3.0)
            bd = t2("bd", c, 3)
            nc.vector.tensor_tensor(
                bd, A[:, 0:3, :],
                qv.rearrange('p (k x) -> p k x', k=1).broadcast_to([pj, 3, fch]),
                OP.subtract)
            sq6 = t2("sq6", c, 6)
            nc.gpsimd.tensor_mul(sq6[:, 0:3, :], bd, bd)
            nc.gpsimd.tensor_mul(sq6[:, 3:6, :], A[:, 3:6, :], A[:, 3:6, :])
            sd = t2("sd", c)
            nc.gpsimd.tensor_add(sd, sq6[:, 0, :], sq6[:, 1, :])
            nc.gpsimd.tensor_add(sd, sd, sq6[:, 2, :])
            so = t2("so", c)
            nc.gpsimd.tensor_add(so, sq6[:, 3, :], sq6[:, 4, :])
            nc.gpsimd.tensor_add(so, so, sq6[:, 5, :])
            p2 = t2("p2", c)
            nc.vector.scalar_tensor_tensor(p2, so, 2.0, sd, OP.mult, OP.add)

            b0, b1, b2 = bd[:, 0, :], bd[:, 1, :], bd[:, 2, :]
            o01, o02, o12 = A[:, 3, :], A[:, 4, :], A[:, 5, :]
            x1 = t2("x1", c); x2 = t2("x2", c); x3 = t2("x3", c)
            det = t2("det", c)
            nc.vector.tensor_mul(x1, b1, b2)
            nc.vector.tensor_sub(x1, x1, sq6[:, 5, :])
            nc.vector.tensor_mul(det, b0, x1)                 # T1
            nc.gpsimd.tensor_mul(x2, o01, b2)
            nc.gpsimd.tensor_mul(x3, o12, o02)
            nc.gpsimd.tensor_sub(x2, x2, x3)
            nc.gpsimd.tensor_mul(x2, x2, o01)                 # T2
            nc.vector.tensor_sub(det, det, x2)
            nc.gpsimd.tensor_mul(x1, o01, o12)
            nc.gpsimd.tensor_mul(x3, b1, o02)
            nc.gpsimd.tensor_sub(x1, x1, x3)
            nc.gpsimd.tensor_mul(x1, x1, o02)                 # T3
            nc.vector.tensor_add(det, det, x1)

            sv = t2("sv", c)
            nc.gpsimd.tensor_scalar_mul(sv, p2, 1.0 / 6.0)
            s2 = t2("s2", c)
            nc.vector.tensor_mul(s2, sv, sv)
            u = t2("u", c)
            nc.vector.scalar_tensor_tensor(u, sv, 4.0, s2, OP.mult, OP.mult)
            dt2 = t2("dt2", c)
            nc.gpsimd.tensor_mul(dt2, det, det)
            nc.gpsimd.tensor_sub(u, u, dt2)
            nc.vector.tensor_scalar_max(u, u, 1e-30)
            ru = t2("ru", c)
            nc.vector.reciprocal_approx_fast(ru, u)
            chunks.append((isl, qv, sv, det, ru))

        # ACT sqrt-family block (both chunks), then trig, then sqrt again
        acts = []
        for c, (isl, qv, sv, det, ru) in enumerate(chunks):
            rs = t2("rs", c)
            nc.scalar.activation(rs, ru, AF.Sqrt, bias=bias0)
            pv = t2("pv", c)
            nc.scalar.activation(pv, sv, AF.Sqrt, bias=bias0)
            arg = t2("arg", c)
            nc.vector.tensor_mul(arg, det, rs)
            aa = t2("aa", c)
            nc.scalar.activation(aa, arg, AF.Abs, bias=bias0)
            sgn = t2("sgn", c)
            nc.scalar.activation(sgn, arg, AF.Sign, bias=bias0)
            am = t2("am", c)
            nc.vector.tensor_scalar_max(am, aa, 1e-20)
            ia = t2("ia", c)
            nc.vector.reciprocal_approx_fast(ia, am)
            mm = t2("mm", c)
            nc.vector.tensor_tensor(mm, am, ia, OP.min)
            acts.append((c, qv, pv, aa, sgn, mm))

        lams = []
        for (c, qv, pv, aa, sgn, mm) in acts:
            at = t2("at", c)
            nc.scalar.activation(at, mm, AF.Arctan, bias=bias0)
            fl = t2("fl", c)
            nc.vector.tensor_scalar(fl, aa, 1.0, 1.0, OP.is_gt, OP.mult)
            f2 = t2("f2", c)
            nc.vector.tensor_scalar(f2, at, -2.0, float(np.pi / 2),
                                    OP.mult, OP.add)
            nc.gpsimd.tensor_mul(fl, fl, f2)
            nc.vector.tensor_add(at, at, fl)
            nc.vector.tensor_mul(at, at, sgn)                 # atan(arg)
            # cs_k = cos(theta + phi_k), theta = pi/6 - at/3, via Sin
            cs = t2("cs", c, 3)
            for k, sc in enumerate((-1.0 / 3.0, -1.0 / 3.0, 1.0 / 3.0)):
                nc.scalar.activation(cs[:, k, :], at, AF.Sin,
                                     bias=sinb[:, k:k + 1], scale=sc)
            tp = t2("tp", c)
            nc.gpsimd.tensor_scalar_mul(tp, pv, 2.0)
            lam = t2("lam", c, 3)
            nc.vector.tensor_tensor(
                lam, cs,
                tp.rearrange('p (k x) -> p k x', k=1).broadcast_to([pj, 3, fch]),
                OP.mult)
            nc.gpsimd.tensor_tensor(
                lam, lam,
                qv.rearrange('p (k x) -> p k x', k=1).broadcast_to([pj, 3, fch]),
                OP.add)
            nc.vector.tensor_scalar_max(lam, lam, 0.0)
            lams.append((c, lam))

        for (c, lam) in lams:
            sgr = t2("sgr", c, 3)
            nc.scalar.activation(sgr, lam, AF.Sqrt, bias=bias0)
            n1 = t2("n1", c)
            nc.gpsimd.tensor_add(n1, sgr[:, 0, :], sgr[:, 1, :])
            nc.gpsimd.tensor_add(n1, n1, sgr[:, 2, :])
            nc.vector.tensor_reduce(outp[:, c:c + 1], n1,
                                    mybir.AxisListType.X, OP.add)

        nc.sync.dma_start(out=out, in_=outp)


# ---------------------------------------------------------------------------
# entry point
# ---------------------------------------------------------------------------

_NC_CACHE = {}


def _get_nc(grid=GRID, cores=CORES):
    key = (grid, cores)
    if key not in _NC_CACHE:
        _NC_CACHE[key] = build_bass(grid, cores)
    return _NC_CACHE[key]


def run_device(g_maps, v_maps, grid=GRID, cores=CORES, trace=False):
    nc = _get_nc(grid, cores)
    mats = _shift_mat(128)
    in_maps = [{"vtx": v_maps[c], "gfc": g_maps[c], "mats": mats}
               for c in range(cores)]
    res = run_bass_kernel_spmd(nc, in_maps, core_ids=list(range(cores)),
                               trace=trace)
    return res


def kernel(V_deformed, elem_rest, elem_weights, elem_idx):
    V = np.asarray(V_deformed, np.float32)
    er = np.asarray(elem_rest, np.float32)
    ew = np.asarray(elem_weights, np.float32)
    ei = np.asarray(elem_idx)
    n = GRID
    assert V.shape == (n * n, 3)

    if not _structure_ok(ei, n):
        return _reference_fallback(V, er, ew, ei)

    g_maps, v_maps, e_r_total = _host_prep(V, er, ew, n, CORES)
    res = run_device(g_maps, v_maps, n, CORES)
    nuc_sum = 0.0
    e_sum = 0.0
    for r in res.results:
        o = r["out"].astype(np.float64)
        nuc_sum += o[:, 0].sum() + o[:, 1].sum()
        e_sum += o[:, 2].sum()
    loss = 3.0 * (e_sum + e_r_total) - 2.0 * nuc_sum
    return np.asarray(loss, dtype=np.float32)
